# revision 16
# baseline (speedup 1.0000x reference)
"""DeepSet (local MLP -> segment_sum -> global MLP) on 8 TRN2 NeuronCores.

Contract: kernel(**inputs) takes FULL inputs, returns FULL [1024, 64] f32 output.

Sharding: `batch` is sorted, so segment b's rows are contiguous. Core k owns
segments [128k, 128(k+1)) and exactly the rows whose batch id falls in that
window (host finds the boundaries with searchsorted). Every core therefore
fully owns its 128 output rows -- no collective at all.

Math: with S1_b = sum_{r in seg b} relu(x_r @ W1 + b1)  (a [128, d_h] tile
per core) and n_b = |seg b|,
    s_b   = S1_b @ W2 + n_b * b2          (matmul associativity)
    out_b = relu(s_b @ W3 + b3) @ W4 + b4
so only the first layer touches the big N-row stream; everything after the
segment reduction is a tiny per-core epilogue.

Per 128-row tile on chip (rows on partitions, "natural" layout):
    h1p  = xT_tile.T @ W1 + ones @ b1          (PE, bf16, PSUM f32)
    h1r  = relu(h1p)                           (ACT, PSUM->SBUF, cast bf16)
    M    = (iota == batch_local)               (DVE/GPSIMD one-hot [rows, seg])
    S1  += M.T @ h1r                           (PE, accumulates in PSUM)
Rows are padded per core to a fixed NROWS; pad rows carry batch_local=200 so
their one-hot row is all zero and they contribute nothing.

Walrus codegen limits TensorScalar to a single sync-wait, so iota and the
batch-local ids ship in ONE f32 `meta` tensor (single DMA -> single wait).
"""

import numpy as np
import ml_dtypes

BF16 = ml_dtypes.bfloat16

N_CORES = 8
B = 1024
SEGS = B // N_CORES  # 128 segments per core
D_IN, D_H, D_RHO, D_OUT = 128, 128, 128, 64
P = 128  # partitions / tile rows

NROWS_DEFAULT = 65536  # padded rows per core (multiple of CHUNK)
CHUNK = 4096  # rows per DMA chunk
PAD_ID = 200.0  # batch_local value for pad rows; never matches iota 0..127

_nc_cache: dict[int, object] = {}


def _build_nc(nrows: int):
    import concourse.bass as bass
    import concourse.bacc as bacc
    import concourse.mybir as mybir
    import concourse.tile as tile

    f32 = mybir.dt.float32
    bf16 = mybir.dt.bfloat16
    Relu = mybir.ActivationFunctionType.Relu
    Copy = mybir.ActivationFunctionType.Copy
    is_eq = mybir.AluOpType.is_equal

    nt = nrows // P  # 128-row tiles per core
    tpc = CHUNK // P  # tiles per DMA chunk
    nch = nrows // CHUNK

    nc = bacc.Bacc()

    xt_d = nc.dram_tensor("xt", [P, nrows], bf16, kind="ExternalInput")
    # meta = batch_local ids, tiled [P, nt]  (bf16; small ints, exact)
    meta_d = nc.dram_tensor("meta", [P, nt], bf16, kind="ExternalInput")
    # iex[p, c*8+j] = c  -- segment-major expanded iota (all stride-1 TT)
    iex_d = nc.dram_tensor("iex", [P, 8 * P], bf16, kind="ExternalInput")
    w1_d = nc.dram_tensor("w1", [D_IN, D_H], bf16, kind="ExternalInput")
    # brow = [b1 x4 | ones]  (bf16 rows for the grouped bias matmul)
    brow_d = nc.dram_tensor("brow", [1, 5 * P], bf16, kind="ExternalInput")
    # cmat = [W2 | W3 | W4 | I]  (f32 epilogue matrices)
    cmat_d = nc.dram_tensor("cmat", [P, 448], f32, kind="ExternalInput")
    # crow = [cnt | b2 | b3 | b4 | ones]  (f32 epilogue rows)
    crow_d = nc.dram_tensor("crow", [1, 576], f32, kind="ExternalInput")
    out_d = nc.dram_tensor("out", [SEGS, D_OUT], f32, kind="ExternalOutput")

    with tile.TileContext(nc) as tc:
        with (
            tc.tile_pool(name="const", bufs=1) as cpool,
            tc.tile_pool(name="xin", bufs=4) as xpool,
            tc.tile_pool(name="hr", bufs=4) as hrpool,
            tc.tile_pool(name="onehot_v", bufs=8) as mpool_v,
            tc.tile_pool(name="fin", bufs=1) as fpool,
            tc.tile_pool(name="ph", bufs=3, space=bass.MemorySpace.PSUM) as phpool,
            tc.tile_pool(name="ps", bufs=1, space=bass.MemorySpace.PSUM) as pspool,
            tc.tile_pool(name="pf", bufs=1, space=bass.MemorySpace.PSUM) as pfpool,
        ):
            # resident constants
            w1 = cpool.tile([D_IN, D_H], bf16, tag="w1")
            nc.sync.dma_start(w1[:], w1_d[:])
            brow = cpool.tile([1, 5 * P], bf16, tag="brow")
            nc.sync.dma_start(brow[:], brow_d[:])
            meta = cpool.tile([P, nt], bf16, tag="meta")
            nc.sync.dma_start(meta[:], meta_d[:])
            iex = cpool.tile([P, 8 * P], bf16, tag="iex")
            nc.sync.dma_start(iex[:], iex_d[:])
            b1rep4 = brow[:, 0:4 * P]
            ones_b = brow[:, 4 * P:5 * P]

            s1p = pspool.tile([SEGS, D_H], f32, tag="s1")

            for ch in range(nch):
                xc = xpool.tile([P, CHUNK], bf16, tag="xc")
                if ch == 0:
                    # split the first chunk so PE can start ~2.5us earlier
                    q = CHUNK // 4
                    for sq in range(4):
                        nc.sync.dma_start(
                            xc[:, sq * q:(sq + 1) * q],
                            xt_d[:, sq * q:(sq + 1) * q])
                else:
                    nc.sync.dma_start(
                        xc[:], xt_d[:, ch * CHUNK:(ch + 1) * CHUNK])
                for g in range(tpc // 8):
                    # seg-major one-hot, all operands stride-1 in last dim
                    # (DVE 2x mode): mtc[p, c*8+j] = (c == bl[p, t0+j])
                    t0 = ch * tpc + g * 8
                    mtc = mpool_v.tile([P, 8 * P], bf16, tag="mtv")
                    iexv = iex[:]
                    a1 = bass.AP(iexv.tensor, iexv.offset,
                                 [[iexv.ap[0][0], P], [8, P], [1, 8]])
                    blsl = meta[:, t0:t0 + 8]
                    a2 = bass.AP(blsl.tensor, blsl.offset,
                                 [[blsl.ap[0][0], P], [0, P], [1, 8]])
                    mtv = mtc[:]
                    a3 = bass.AP(mtv.tensor, mtv.offset,
                                 [[mtv.ap[0][0], P], [8, P], [1, 8]])
                    nc.vector.tensor_tensor(a3, a1, a2, is_eq)
                    h1p = phpool.tile([P, 8 * D_H], f32, tag="h1p")
                    for hlf in range(2):
                        nc.tensor.matmul(
                            h1p[:, hlf * 512:(hlf + 1) * 512], ones_b, b1rep4,
                            start=True, stop=False, skip_group_check=True,
                        )
                    for j8 in range(8):
                        tic = g * 8 + j8
                        nc.tensor.matmul(
                            h1p[:, j8 * P:(j8 + 1) * P],
                            xc[:, tic * P:(tic + 1) * P], w1[:],
                            start=False, stop=(j8 == 7), skip_group_check=True,
                        )
                    h1r = hrpool.tile([P, 8 * D_H], bf16, tag="h1r")
                    nc.scalar.activation(h1r[:], h1p[:], Relu)
                    for j8 in range(8):
                        tic = g * 8 + j8
                        t = ch * tpc + tic
                        nc.tensor.matmul(
                            s1p[:], mtc[:, j8::8],
                            h1r[:, j8 * P:(j8 + 1) * P],
                            start=(t == 0), stop=(t == nt - 1),
                            skip_group_check=True,
                        )

            # ---- per-core epilogue (all f32, tiny) ----
            cmat = cpool.tile([P, 448], f32, tag="cmat")
            nc.sync.dma_start(cmat[:], cmat_d[:])
            crow = cpool.tile([1, 576], f32, tag="crow")
            nc.sync.dma_start(crow[:], crow_d[:])
            w2 = cmat[:, 0:128]
            w3 = cmat[:, 128:256]
            w4 = cmat[:, 256:320]
            ident = cmat[:, 320:448]
            cntr = crow[:, 0:128]
            b2r = crow[:, 128:256]
            b3r = crow[:, 256:384]
            b4r = crow[:, 384:448]
            ones_f = crow[:, 448:576]

            s1s = fpool.tile([SEGS, D_H], f32, tag="s1s")
            nc.scalar.activation(s1s[:], s1p[:], Copy)

            # s2 = S1 @ W2 + counts x b2
            tp1 = pfpool.tile([D_H, SEGS], f32, tag="fmm")
            nc.tensor.transpose(tp1[:], s1s[:], ident)
            s1t = fpool.tile([D_H, SEGS], f32, tag="s1t")
            nc.scalar.activation(s1t[:], tp1[:], Copy)
            s2p = pfpool.tile([SEGS, D_RHO], f32, tag="fmm")
            nc.tensor.matmul(s2p[:], s1t[:], w2,
                             start=True, stop=False, skip_group_check=True)
            nc.tensor.matmul(s2p[:], cntr, b2r,
                             start=False, stop=True, skip_group_check=True)
            s2s = fpool.tile([SEGS, D_RHO], f32, tag="s2s")
            nc.scalar.activation(s2s[:], s2p[:], Copy)

            # g1 = relu(s2 @ W3 + b3)
            tp2 = pfpool.tile([D_RHO, SEGS], f32, tag="fmm")
            nc.tensor.transpose(tp2[:], s2s[:], ident)
            s2t = fpool.tile([D_RHO, SEGS], f32, tag="s2t")
            nc.scalar.activation(s2t[:], tp2[:], Copy)
            g1p = pfpool.tile([SEGS, D_H], f32, tag="fmm")
            nc.tensor.matmul(g1p[:], s2t[:], w3,
                             start=True, stop=False, skip_group_check=True)
            nc.tensor.matmul(g1p[:], ones_f, b3r,
                             start=False, stop=True, skip_group_check=True)
            g1s = fpool.tile([SEGS, D_H], f32, tag="g1s")
            nc.scalar.activation(g1s[:], g1p[:], Relu)

            # out = g1 @ W4 + b4
            tp3 = pfpool.tile([D_H, SEGS], f32, tag="fmm")
            nc.tensor.transpose(tp3[:], g1s[:], ident)
            g1t = fpool.tile([D_H, SEGS], f32, tag="g1t")
            nc.scalar.activation(g1t[:], tp3[:], Copy)
            outp = pfpool.tile([SEGS, D_OUT], f32, tag="fmm")
            nc.tensor.matmul(outp[:], g1t[:], w4,
                             start=True, stop=False, skip_group_check=True)
            nc.tensor.matmul(outp[:], ones_f, b4r[:, 0:D_OUT],
                             start=False, stop=True, skip_group_check=True)
            outs = fpool.tile([SEGS, D_OUT], f32, tag="outs")
            nc.scalar.activation(outs[:], outp[:], Copy)
            nc.sync.dma_start(out_d[:], outs[:])

    nc.finalize()
    return nc


def _get_nc(nrows: int):
    if nrows not in _nc_cache:
        _nc_cache[nrows] = _build_nc(nrows)
    return _nc_cache[nrows]


def _prep_in_maps(x, batch, W1, b1, W2, b2, W3, b3, W4, b4):
    """Host-side sharding/packing. Returns (nrows, in_maps)."""
    x = np.asarray(x, dtype=np.float32)
    batch = np.asarray(batch)
    W1 = np.asarray(W1, dtype=np.float32)
    b1 = np.asarray(b1, dtype=np.float32)
    W2 = np.asarray(W2, dtype=np.float32)
    b2 = np.asarray(b2, dtype=np.float32)
    W3 = np.asarray(W3, dtype=np.float32)
    b3 = np.asarray(b3, dtype=np.float32)
    W4 = np.asarray(W4, dtype=np.float32)
    b4 = np.asarray(b4, dtype=np.float32)

    bounds = np.searchsorted(batch, np.arange(0, B + 1, SEGS), side="left")
    counts_all = np.bincount(batch.astype(np.int64), minlength=B).astype(np.float32)

    max_rows = int(np.max(bounds[1:] - bounds[:-1]))
    nrows = NROWS_DEFAULT
    while nrows < max_rows:
        nrows += CHUNK
    nt = nrows // P

    xbf = x.astype(BF16)
    iex_np = np.repeat(np.arange(P, dtype=np.float32), 8)[None, :].repeat(P, 0)
    iex_np = np.ascontiguousarray(iex_np).astype(BF16)

    cmat = np.concatenate([W2, W3, W4, np.eye(P, dtype=np.float32)],
                          axis=1).astype(np.float32)
    brow = np.concatenate([np.tile(b1, 4), np.ones(P, np.float32)]
                          ).reshape(1, -1).astype(BF16)

    in_maps = []
    for k in range(N_CORES):
        lo, hi = int(bounds[k]), int(bounds[k + 1])
        cnt_rows = hi - lo
        xt = np.zeros((P, nrows), dtype=BF16)
        if cnt_rows:
            xt[:, :cnt_rows] = xbf[lo:hi].T
        blv = np.full(nrows, PAD_ID, dtype=np.float32)
        if cnt_rows:
            blv[:cnt_rows] = (batch[lo:hi] - k * SEGS).astype(np.float32)
        meta = np.ascontiguousarray(blv.reshape(nt, P).T).astype(BF16)
        crow = np.concatenate([
            counts_all[k * SEGS:(k + 1) * SEGS],
            b2, b3, b4, np.ones(P, np.float32),
        ]).reshape(1, -1).astype(np.float32)
        in_maps.append({
            "xt": xt, "meta": meta, "iex": iex_np,
            "w1": W1.astype(BF16),
            "brow": brow, "cmat": cmat, "crow": crow,
        })
    return nrows, in_maps


def kernel(x, batch, W1, b1, W2, b2, W3, b3, W4, b4):
    from concourse.bass_utils import run_bass_kernel_spmd

    nrows, in_maps = _prep_in_maps(x, batch, W1, b1, W2, b2, W3, b3, W4, b4)
    nc = _get_nc(nrows)
    res = run_bass_kernel_spmd(nc, in_maps, list(range(N_CORES)))
    out = np.concatenate([r["out"] for r in res.results], axis=0)
    return out.astype(np.float32)


# revision 20
# speedup vs baseline: 1.1425x; 1.1425x over previous
"""DeepSet (local MLP -> segment_sum -> global MLP) on 8 TRN2 NeuronCores.

Contract: kernel(**inputs) takes FULL inputs, returns FULL [1024, 64] f32 output.

Sharding: `batch` is sorted, so segment b's rows are contiguous. Core k owns
segments [128k, 128(k+1)) and exactly the rows whose batch id falls in that
window (host finds the boundaries with searchsorted). Every core therefore
fully owns its 128 output rows -- no collective at all.

Math: with S1_b = sum_{r in seg b} relu(x_r @ W1 + b1)  (a [128, d_h] tile
per core) and n_b = |seg b|,
    s_b   = S1_b @ W2 + n_b * b2          (matmul associativity)
    out_b = relu(s_b @ W3 + b3) @ W4 + b4
so only the first layer touches the big N-row stream; everything after the
segment reduction is a tiny per-core epilogue.

Per 8-tile group (1024 rows) on chip, rows on partitions ("natural" layout):
    h1p  = ones @ b1 (PSUM init) + xT_tile.T @ W1 per tile   (PE, bf16)
    h1r  = relu(h1p)                 (one ACT op per group, PSUM->SBUF, bf16)
    M    = (iex == batch_local)      (one DVE tensor_tensor per group; the
                                      segment-major layout keeps every operand
                                      stride-1 in the last dim -> DVE 2x mode)
    S1  += M_tile.T @ h1r_tile       (PE, 512-matmul PSUM accumulation)
Rows are padded per core to a fixed NROWS; pad rows carry batch_local=200 so
their one-hot row is all zero and they contribute nothing. Built with Bacc so
wait-count legalization (event semaphores) runs before walrus codegen.
"""

import numpy as np
import ml_dtypes

BF16 = ml_dtypes.bfloat16

N_CORES = 8
B = 1024
SEGS = B // N_CORES  # 128 segments per core
D_IN, D_H, D_RHO, D_OUT = 128, 128, 128, 64
P = 128  # partitions / tile rows

NROWS_DEFAULT = 65536  # padded rows per core (multiple of CHUNK)
CHUNK = 4096  # rows per DMA chunk
PAD_ID = 200.0  # batch_local value for pad rows; never matches iota 0..127

_nc_cache: dict[int, object] = {}


def _build_nc(nrows: int):
    import concourse.bass as bass
    import concourse.bacc as bacc
    import concourse.mybir as mybir
    import concourse.tile as tile

    f32 = mybir.dt.float32
    bf16 = mybir.dt.bfloat16
    Relu = mybir.ActivationFunctionType.Relu
    Copy = mybir.ActivationFunctionType.Copy
    is_eq = mybir.AluOpType.is_equal

    nt = nrows // P  # 128-row tiles per core
    tpc = CHUNK // P  # tiles per DMA chunk
    nch = nrows // CHUNK

    nc = bacc.Bacc()

    xt_d = nc.dram_tensor("xt", [P, nrows], bf16, kind="ExternalInput")
    # meta = batch_local ids, tiled [P, nt]  (bf16; small ints, exact)
    meta_d = nc.dram_tensor("meta", [P, nt], bf16, kind="ExternalInput")
    # iex[p, c*8+j] = c  -- segment-major expanded iota (all stride-1 TT)
    iex_d = nc.dram_tensor("iex", [P, 8 * P], bf16, kind="ExternalInput")
    w1_d = nc.dram_tensor("w1", [D_IN, D_H], bf16, kind="ExternalInput")
    # cmat = [W2 | W3 | W4 | I]  (f32 epilogue matrices)
    cmat_d = nc.dram_tensor("cmat", [P, 448], f32, kind="ExternalInput")
    # crow = [cnt | b2 | b3 | b4 | ones]  (f32 epilogue rows)
    crow_d = nc.dram_tensor("crow", [1, 576], f32, kind="ExternalInput")
    fp8 = mybir.dt.float8e4
    # b8 = [b1rep4_hi(512) | b1rep4_lo(512) | ones(256)] fp8 for the
    # DoubleRow bias matmul (0.5 cy/row); hi/lo residual split keeps b1
    # accurate to ~1e-3.
    b8_d = nc.dram_tensor("b8", [1, 1280], fp8, kind="ExternalInput")
    out_d = nc.dram_tensor("out", [SEGS, D_OUT], f32, kind="ExternalOutput")

    with tile.TileContext(nc) as tc:
        with (
            tc.tile_pool(name="const", bufs=1) as cpool,
            tc.tile_pool(name="xin", bufs=4) as xpool,
            tc.tile_pool(name="hr", bufs=4) as hrpool,
            tc.tile_pool(name="onehot_v", bufs=8) as mpool_v,
            tc.tile_pool(name="fin", bufs=1) as fpool,
            tc.tile_pool(name="ph", bufs=3, space=bass.MemorySpace.PSUM) as phpool,
            tc.tile_pool(name="ps", bufs=1, space=bass.MemorySpace.PSUM) as pspool,
            tc.tile_pool(name="pf", bufs=1, space=bass.MemorySpace.PSUM) as pfpool,
        ):
            # resident constants
            w1 = cpool.tile([D_IN, D_H], bf16, tag="w1")
            nc.sync.dma_start(w1[:], w1_d[:])
            meta = cpool.tile([P, nt], bf16, tag="meta")
            nc.sync.dma_start(meta[:], meta_d[:])
            iex = cpool.tile([P, 8 * P], bf16, tag="iex")
            nc.sync.dma_start(iex[:], iex_d[:])
            b8 = cpool.tile([1, 1280], fp8, tag="b8")
            nc.sync.dma_start(b8[:], b8_d[:])
            b8v = b8[:]
            bias_rhs = bass.AP(b8v.tensor, b8v.offset,
                               [[b8v.ap[0][0], 1], [512, 2], [1, 512]])
            bias_lhs = bass.AP(b8v.tensor, b8v.offset + 1024,
                               [[b8v.ap[0][0], 1], [128, 2], [1, 128]])

            s1p = pspool.tile([SEGS, D_H], f32, tag="s1")

            for ch in range(nch):
                xc = xpool.tile([P, CHUNK], bf16, tag="xc")
                if ch == 0:
                    # split the first chunk so PE can start ~2.5us earlier
                    q = CHUNK // 4
                    for sq in range(4):
                        nc.sync.dma_start(
                            xc[:, sq * q:(sq + 1) * q],
                            xt_d[:, sq * q:(sq + 1) * q])
                else:
                    nc.sync.dma_start(
                        xc[:], xt_d[:, ch * CHUNK:(ch + 1) * CHUNK])
                for g in range(tpc // 8):
                    # seg-major one-hot, all operands stride-1 in last dim
                    # (DVE 2x mode): mtc[p, c*8+j] = (c == bl[p, t0+j])
                    t0 = ch * tpc + g * 8
                    mtc = mpool_v.tile([P, 8 * P], bf16, tag="mtv")
                    iexv = iex[:]
                    a1 = bass.AP(iexv.tensor, iexv.offset,
                                 [[iexv.ap[0][0], P], [8, P], [1, 8]])
                    blsl = meta[:, t0:t0 + 8]
                    a2 = bass.AP(blsl.tensor, blsl.offset,
                                 [[blsl.ap[0][0], P], [0, P], [1, 8]])
                    mtv = mtc[:]
                    a3 = bass.AP(mtv.tensor, mtv.offset,
                                 [[mtv.ap[0][0], P], [8, P], [1, 8]])
                    nc.vector.tensor_tensor(a3, a1, a2, is_eq)
                    h1p = phpool.tile([P, 8 * D_H], f32, tag="h1p")
                    for hlf in range(2):
                        nc.tensor.matmul(
                            h1p[:, hlf * 512:(hlf + 1) * 512],
                            bias_lhs, bias_rhs,
                            start=True, stop=False,
                            perf_mode=mybir.MatmulPerfMode.DoubleRow,
                            skip_group_check=True,
                        )
                    for j8 in range(8):
                        tic = g * 8 + j8
                        nc.tensor.matmul(
                            h1p[:, j8 * P:(j8 + 1) * P],
                            xc[:, tic * P:(tic + 1) * P], w1[:],
                            start=False, stop=(j8 == 7), skip_group_check=True,
                        )
                    h1r = hrpool.tile([P, 8 * D_H], bf16, tag="h1r")
                    nc.scalar.activation(h1r[:], h1p[:], Relu)
                    for j8 in range(8):
                        tic = g * 8 + j8
                        t = ch * tpc + tic
                        nc.tensor.matmul(
                            s1p[:], mtc[:, j8::8],
                            h1r[:, j8 * P:(j8 + 1) * P],
                            start=(t == 0), stop=(t == nt - 1),
                            skip_group_check=True,
                        )

            # ---- per-core epilogue (all f32, tiny) ----
            cmat = cpool.tile([P, 448], f32, tag="cmat")
            nc.sync.dma_start(cmat[:], cmat_d[:])
            crow = cpool.tile([1, 576], f32, tag="crow")
            nc.sync.dma_start(crow[:], crow_d[:])
            w2 = cmat[:, 0:128]
            w3 = cmat[:, 128:256]
            w4 = cmat[:, 256:320]
            ident = cmat[:, 320:448]
            cntr = crow[:, 0:128]
            b2r = crow[:, 128:256]
            b3r = crow[:, 256:384]
            b4r = crow[:, 384:448]
            ones_f = crow[:, 448:576]

            s1s = fpool.tile([SEGS, D_H], f32, tag="s1s")
            nc.scalar.activation(s1s[:], s1p[:], Copy)

            # s2 = S1 @ W2 + counts x b2
            tp1 = pfpool.tile([D_H, SEGS], f32, tag="fmm")
            nc.tensor.transpose(tp1[:], s1s[:], ident)
            s1t = fpool.tile([D_H, SEGS], f32, tag="s1t")
            nc.scalar.activation(s1t[:], tp1[:], Copy)
            s2p = pfpool.tile([SEGS, D_RHO], f32, tag="fmm")
            nc.tensor.matmul(s2p[:], s1t[:], w2,
                             start=True, stop=False, skip_group_check=True)
            nc.tensor.matmul(s2p[:], cntr, b2r,
                             start=False, stop=True, skip_group_check=True)
            s2s = fpool.tile([SEGS, D_RHO], f32, tag="s2s")
            nc.scalar.activation(s2s[:], s2p[:], Copy)

            # g1 = relu(s2 @ W3 + b3)
            tp2 = pfpool.tile([D_RHO, SEGS], f32, tag="fmm")
            nc.tensor.transpose(tp2[:], s2s[:], ident)
            s2t = fpool.tile([D_RHO, SEGS], f32, tag="s2t")
            nc.scalar.activation(s2t[:], tp2[:], Copy)
            g1p = pfpool.tile([SEGS, D_H], f32, tag="fmm")
            nc.tensor.matmul(g1p[:], s2t[:], w3,
                             start=True, stop=False, skip_group_check=True)
            nc.tensor.matmul(g1p[:], ones_f, b3r,
                             start=False, stop=True, skip_group_check=True)
            g1s = fpool.tile([SEGS, D_H], f32, tag="g1s")
            nc.scalar.activation(g1s[:], g1p[:], Relu)

            # out = g1 @ W4 + b4
            tp3 = pfpool.tile([D_H, SEGS], f32, tag="fmm")
            nc.tensor.transpose(tp3[:], g1s[:], ident)
            g1t = fpool.tile([D_H, SEGS], f32, tag="g1t")
            nc.scalar.activation(g1t[:], tp3[:], Copy)
            outp = pfpool.tile([SEGS, D_OUT], f32, tag="fmm")
            nc.tensor.matmul(outp[:], g1t[:], w4,
                             start=True, stop=False, skip_group_check=True)
            nc.tensor.matmul(outp[:], ones_f, b4r[:, 0:D_OUT],
                             start=False, stop=True, skip_group_check=True)
            outs = fpool.tile([SEGS, D_OUT], f32, tag="outs")
            nc.scalar.activation(outs[:], outp[:], Copy)
            nc.sync.dma_start(out_d[:], outs[:])

    nc.finalize()
    return nc


def _get_nc(nrows: int):
    if nrows not in _nc_cache:
        _nc_cache[nrows] = _build_nc(nrows)
    return _nc_cache[nrows]


def _prep_in_maps(x, batch, W1, b1, W2, b2, W3, b3, W4, b4):
    """Host-side sharding/packing. Returns (nrows, in_maps)."""
    x = np.asarray(x, dtype=np.float32)
    batch = np.asarray(batch)
    W1 = np.asarray(W1, dtype=np.float32)
    b1 = np.asarray(b1, dtype=np.float32)
    W2 = np.asarray(W2, dtype=np.float32)
    b2 = np.asarray(b2, dtype=np.float32)
    W3 = np.asarray(W3, dtype=np.float32)
    b3 = np.asarray(b3, dtype=np.float32)
    W4 = np.asarray(W4, dtype=np.float32)
    b4 = np.asarray(b4, dtype=np.float32)

    bounds = np.searchsorted(batch, np.arange(0, B + 1, SEGS), side="left")
    counts_all = np.bincount(batch.astype(np.int64), minlength=B).astype(np.float32)

    max_rows = int(np.max(bounds[1:] - bounds[:-1]))
    nrows = NROWS_DEFAULT
    while nrows < max_rows:
        nrows += CHUNK
    nt = nrows // P

    xbf = x.astype(BF16)
    iex_np = np.repeat(np.arange(P, dtype=np.float32), 8)[None, :].repeat(P, 0)
    iex_np = np.ascontiguousarray(iex_np).astype(BF16)

    cmat = np.concatenate([W2, W3, W4, np.eye(P, dtype=np.float32)],
                          axis=1).astype(np.float32)
    FP8 = ml_dtypes.float8_e4m3fn
    b1rep = np.tile(b1, 4).astype(np.float32)
    b1hi = b1rep.astype(FP8)
    b1lo = (b1rep - b1hi.astype(np.float32)).astype(FP8)
    b8 = np.concatenate([np.asarray(b1hi), np.asarray(b1lo),
                         np.ones(256, np.float32).astype(FP8)]).reshape(1, -1)

    in_maps = []
    for k in range(N_CORES):
        lo, hi = int(bounds[k]), int(bounds[k + 1])
        cnt_rows = hi - lo
        xt = np.zeros((P, nrows), dtype=BF16)
        if cnt_rows:
            xt[:, :cnt_rows] = xbf[lo:hi].T
        blv = np.full(nrows, PAD_ID, dtype=np.float32)
        if cnt_rows:
            blv[:cnt_rows] = (batch[lo:hi] - k * SEGS).astype(np.float32)
        meta = np.ascontiguousarray(blv.reshape(nt, P).T).astype(BF16)
        crow = np.concatenate([
            counts_all[k * SEGS:(k + 1) * SEGS],
            b2, b3, b4, np.ones(P, np.float32),
        ]).reshape(1, -1).astype(np.float32)
        in_maps.append({
            "xt": xt, "meta": meta, "iex": iex_np,
            "w1": W1.astype(BF16),
            "b8": b8, "cmat": cmat, "crow": crow,
        })
    return nrows, in_maps


def kernel(x, batch, W1, b1, W2, b2, W3, b3, W4, b4):
    from concourse.bass_utils import run_bass_kernel_spmd

    nrows, in_maps = _prep_in_maps(x, batch, W1, b1, W2, b2, W3, b3, W4, b4)
    nc = _get_nc(nrows)
    res = run_bass_kernel_spmd(nc, in_maps, list(range(N_CORES)))
    out = np.concatenate([r["out"] for r in res.results], axis=0)
    return out.astype(np.float32)


# revision 21
# speedup vs baseline: 1.1522x; 1.0085x over previous
"""DeepSet (local MLP -> segment_sum -> global MLP) on 8 TRN2 NeuronCores.

Contract: kernel(**inputs) takes FULL inputs, returns FULL [1024, 64] f32 output.

Sharding: `batch` is sorted, so segment b's rows are contiguous. Core k owns
segments [128k, 128(k+1)) and exactly the rows whose batch id falls in that
window (host finds the boundaries with searchsorted). Every core therefore
fully owns its 128 output rows -- no collective at all.

Math: with S1_b = sum_{r in seg b} relu(x_r @ W1 + b1)  (a [128, d_h] tile
per core) and n_b = |seg b|,
    s_b   = S1_b @ W2 + n_b * b2          (matmul associativity)
    out_b = relu(s_b @ W3 + b3) @ W4 + b4
so only the first layer touches the big N-row stream; everything after the
segment reduction is a tiny per-core epilogue.

Per 8-tile group (1024 rows) on chip, rows on partitions ("natural" layout):
    h1p  = ones @ b1 (PSUM init) + xT_tile.T @ W1 per tile   (PE, bf16)
    h1r  = relu(h1p)                 (one ACT op per group, PSUM->SBUF, bf16)
    M    = (iex == batch_local)      (one DVE tensor_tensor per group; the
                                      segment-major layout keeps every operand
                                      stride-1 in the last dim -> DVE 2x mode)
    S1  += M_tile.T @ h1r_tile       (PE, 512-matmul PSUM accumulation)
Rows are padded per core to a fixed NROWS; pad rows carry batch_local=200 so
their one-hot row is all zero and they contribute nothing. Built with Bacc so
wait-count legalization (event semaphores) runs before walrus codegen.
"""

import numpy as np
import ml_dtypes

BF16 = ml_dtypes.bfloat16

N_CORES = 8
B = 1024
SEGS = B // N_CORES  # 128 segments per core
D_IN, D_H, D_RHO, D_OUT = 128, 128, 128, 64
P = 128  # partitions / tile rows

NROWS_DEFAULT = 65536  # padded rows per core (multiple of CHUNK)
CHUNK = 4096  # rows per DMA chunk
PAD_ID = 200.0  # batch_local value for pad rows; never matches iota 0..127

_nc_cache: dict[int, object] = {}


def _build_nc(nrows: int):
    import concourse.bass as bass
    import concourse.bacc as bacc
    import concourse.mybir as mybir
    import concourse.tile as tile

    f32 = mybir.dt.float32
    bf16 = mybir.dt.bfloat16
    Relu = mybir.ActivationFunctionType.Relu
    Copy = mybir.ActivationFunctionType.Copy
    is_eq = mybir.AluOpType.is_equal

    nt = nrows // P  # 128-row tiles per core
    tpc = CHUNK // P  # tiles per DMA chunk
    nch = nrows // CHUNK

    nc = bacc.Bacc()

    xt_d = nc.dram_tensor("xt", [P, nrows], bf16, kind="ExternalInput")
    # meta = batch_local ids, tiled [P, nt]  (bf16; small ints, exact)
    meta_d = nc.dram_tensor("meta", [P, nt], bf16, kind="ExternalInput")
    # iex[p, c*8+j] = c  -- segment-major expanded iota (all stride-1 TT)
    iex_d = nc.dram_tensor("iex", [P, 8 * P], bf16, kind="ExternalInput")
    w1_d = nc.dram_tensor("w1", [D_IN, D_H], bf16, kind="ExternalInput")
    # cmat = [W2 | W3 | W4 | I]  (f32 epilogue matrices)
    cmat_d = nc.dram_tensor("cmat", [P, 448], f32, kind="ExternalInput")
    # crow = [cnt | b2 | b3 | b4 | ones]  (f32 epilogue rows)
    crow_d = nc.dram_tensor("crow", [1, 576], f32, kind="ExternalInput")
    fp8 = mybir.dt.float8e4
    # b8 = [b1rep4_hi(512) | b1rep4_lo(512) | ones(256)] fp8 for the
    # DoubleRow bias matmul (0.5 cy/row); hi/lo residual split keeps b1
    # accurate to ~1e-3.
    b8_d = nc.dram_tensor("b8", [1, 1280], fp8, kind="ExternalInput")
    out_d = nc.dram_tensor("out", [SEGS, D_OUT], f32, kind="ExternalOutput")

    with tile.TileContext(nc) as tc:
        with (
            tc.tile_pool(name="const", bufs=1) as cpool,
            tc.tile_pool(name="xin", bufs=4) as xpool,
            tc.tile_pool(name="hr", bufs=4) as hrpool,
            tc.tile_pool(name="onehot_v", bufs=8) as mpool_v,
            tc.tile_pool(name="fin", bufs=1) as fpool,
            tc.tile_pool(name="ph", bufs=3, space=bass.MemorySpace.PSUM) as phpool,
            tc.tile_pool(name="ps", bufs=1, space=bass.MemorySpace.PSUM) as pspool,
            tc.tile_pool(name="pf", bufs=1, space=bass.MemorySpace.PSUM) as pfpool,
        ):
            # resident constants
            w1 = cpool.tile([D_IN, D_H], bf16, tag="w1")
            nc.sync.dma_start(w1[:], w1_d[:])
            meta = cpool.tile([P, nt], bf16, tag="meta")
            nc.sync.dma_start(meta[:], meta_d[:])
            iex = cpool.tile([P, 8 * P], bf16, tag="iex")
            nc.sync.dma_start(iex[:], iex_d[:])
            b8 = cpool.tile([1, 1280], fp8, tag="b8")
            nc.sync.dma_start(b8[:], b8_d[:])
            b8v = b8[:]
            bias_rhs = bass.AP(b8v.tensor, b8v.offset,
                               [[b8v.ap[0][0], 1], [512, 2], [1, 512]])
            bias_lhs = bass.AP(b8v.tensor, b8v.offset + 1024,
                               [[b8v.ap[0][0], 1], [128, 2], [1, 128]])

            s1p = pspool.tile([SEGS, D_H], f32, tag="s1")

            for ch in range(nch):
                xc = xpool.tile([P, CHUNK], bf16, tag="xc")
                if ch == 0:
                    # split the first chunk so PE can start ~2.5us earlier
                    q = CHUNK // 4
                    for sq in range(4):
                        nc.sync.dma_start(
                            xc[:, sq * q:(sq + 1) * q],
                            xt_d[:, sq * q:(sq + 1) * q])
                else:
                    nc.sync.dma_start(
                        xc[:], xt_d[:, ch * CHUNK:(ch + 1) * CHUNK])
                for g in range(tpc // 8):
                    # seg-major one-hot, all operands stride-1 in last dim
                    # (DVE 2x mode): mtc[p, c*8+j] = (c == bl[p, t0+j])
                    t0 = ch * tpc + g * 8
                    mtc = mpool_v.tile([P, 8 * P], bf16, tag="mtv")
                    iexv = iex[:]
                    a1 = bass.AP(iexv.tensor, iexv.offset,
                                 [[iexv.ap[0][0], P], [8, P], [1, 8]])
                    blsl = meta[:, t0:t0 + 8]
                    a2 = bass.AP(blsl.tensor, blsl.offset,
                                 [[blsl.ap[0][0], P], [0, P], [1, 8]])
                    mtv = mtc[:]
                    a3 = bass.AP(mtv.tensor, mtv.offset,
                                 [[mtv.ap[0][0], P], [8, P], [1, 8]])
                    nc.vector.tensor_tensor(a3, a1, a2, is_eq)
                    h1p = phpool.tile([P, 8 * D_H], f32, tag="h1p")
                    for hlf in range(2):
                        nc.tensor.matmul(
                            h1p[:, hlf * 512:(hlf + 1) * 512],
                            bias_lhs, bias_rhs,
                            start=True, stop=False,
                            perf_mode=mybir.MatmulPerfMode.DoubleRow,
                            skip_group_check=True,
                        )
                    for j8 in range(8):
                        tic = g * 8 + j8
                        nc.tensor.matmul(
                            h1p[:, j8 * P:(j8 + 1) * P],
                            xc[:, tic * P:(tic + 1) * P], w1[:],
                            start=False, stop=(j8 == 7), skip_group_check=True,
                        )
                    h1r = hrpool.tile([P, 8 * D_H], bf16, tag="h1r")
                    gidx = ch * (tpc // 8) + g
                    if gidx % 4 == 3:
                        # balance the PSUM->SBUF relu pass: every 4th group
                        # runs on DVE (max with 0), the rest on ACT
                        nc.vector.tensor_scalar_max(h1r[:], h1p[:], 0.0)
                    else:
                        nc.scalar.activation(h1r[:], h1p[:], Relu)
                    for j8 in range(8):
                        tic = g * 8 + j8
                        t = ch * tpc + tic
                        nc.tensor.matmul(
                            s1p[:], mtc[:, j8::8],
                            h1r[:, j8 * P:(j8 + 1) * P],
                            start=(t == 0), stop=(t == nt - 1),
                            skip_group_check=True,
                        )

            # ---- per-core epilogue (all f32, tiny) ----
            cmat = cpool.tile([P, 448], f32, tag="cmat")
            nc.sync.dma_start(cmat[:], cmat_d[:])
            crow = cpool.tile([1, 576], f32, tag="crow")
            nc.sync.dma_start(crow[:], crow_d[:])
            w2 = cmat[:, 0:128]
            w3 = cmat[:, 128:256]
            w4 = cmat[:, 256:320]
            ident = cmat[:, 320:448]
            cntr = crow[:, 0:128]
            b2r = crow[:, 128:256]
            b3r = crow[:, 256:384]
            b4r = crow[:, 384:448]
            ones_f = crow[:, 448:576]

            s1s = fpool.tile([SEGS, D_H], f32, tag="s1s")
            nc.scalar.activation(s1s[:], s1p[:], Copy)

            # s2 = S1 @ W2 + counts x b2
            tp1 = pfpool.tile([D_H, SEGS], f32, tag="fmm")
            nc.tensor.transpose(tp1[:], s1s[:], ident)
            s1t = fpool.tile([D_H, SEGS], f32, tag="s1t")
            nc.scalar.activation(s1t[:], tp1[:], Copy)
            s2p = pfpool.tile([SEGS, D_RHO], f32, tag="fmm")
            nc.tensor.matmul(s2p[:], s1t[:], w2,
                             start=True, stop=False, skip_group_check=True)
            nc.tensor.matmul(s2p[:], cntr, b2r,
                             start=False, stop=True, skip_group_check=True)
            s2s = fpool.tile([SEGS, D_RHO], f32, tag="s2s")
            nc.scalar.activation(s2s[:], s2p[:], Copy)

            # g1 = relu(s2 @ W3 + b3)
            tp2 = pfpool.tile([D_RHO, SEGS], f32, tag="fmm")
            nc.tensor.transpose(tp2[:], s2s[:], ident)
            s2t = fpool.tile([D_RHO, SEGS], f32, tag="s2t")
            nc.scalar.activation(s2t[:], tp2[:], Copy)
            g1p = pfpool.tile([SEGS, D_H], f32, tag="fmm")
            nc.tensor.matmul(g1p[:], s2t[:], w3,
                             start=True, stop=False, skip_group_check=True)
            nc.tensor.matmul(g1p[:], ones_f, b3r,
                             start=False, stop=True, skip_group_check=True)
            g1s = fpool.tile([SEGS, D_H], f32, tag="g1s")
            nc.scalar.activation(g1s[:], g1p[:], Relu)

            # out = g1 @ W4 + b4
            tp3 = pfpool.tile([D_H, SEGS], f32, tag="fmm")
            nc.tensor.transpose(tp3[:], g1s[:], ident)
            g1t = fpool.tile([D_H, SEGS], f32, tag="g1t")
            nc.scalar.activation(g1t[:], tp3[:], Copy)
            outp = pfpool.tile([SEGS, D_OUT], f32, tag="fmm")
            nc.tensor.matmul(outp[:], g1t[:], w4,
                             start=True, stop=False, skip_group_check=True)
            nc.tensor.matmul(outp[:], ones_f, b4r[:, 0:D_OUT],
                             start=False, stop=True, skip_group_check=True)
            outs = fpool.tile([SEGS, D_OUT], f32, tag="outs")
            nc.scalar.activation(outs[:], outp[:], Copy)
            nc.sync.dma_start(out_d[:], outs[:])

    nc.finalize()
    return nc


def _get_nc(nrows: int):
    if nrows not in _nc_cache:
        _nc_cache[nrows] = _build_nc(nrows)
    return _nc_cache[nrows]


def _prep_in_maps(x, batch, W1, b1, W2, b2, W3, b3, W4, b4):
    """Host-side sharding/packing. Returns (nrows, in_maps)."""
    x = np.asarray(x, dtype=np.float32)
    batch = np.asarray(batch)
    W1 = np.asarray(W1, dtype=np.float32)
    b1 = np.asarray(b1, dtype=np.float32)
    W2 = np.asarray(W2, dtype=np.float32)
    b2 = np.asarray(b2, dtype=np.float32)
    W3 = np.asarray(W3, dtype=np.float32)
    b3 = np.asarray(b3, dtype=np.float32)
    W4 = np.asarray(W4, dtype=np.float32)
    b4 = np.asarray(b4, dtype=np.float32)

    bounds = np.searchsorted(batch, np.arange(0, B + 1, SEGS), side="left")
    counts_all = np.bincount(batch.astype(np.int64), minlength=B).astype(np.float32)

    max_rows = int(np.max(bounds[1:] - bounds[:-1]))
    nrows = NROWS_DEFAULT
    while nrows < max_rows:
        nrows += CHUNK
    nt = nrows // P

    xbf = x.astype(BF16)
    iex_np = np.repeat(np.arange(P, dtype=np.float32), 8)[None, :].repeat(P, 0)
    iex_np = np.ascontiguousarray(iex_np).astype(BF16)

    cmat = np.concatenate([W2, W3, W4, np.eye(P, dtype=np.float32)],
                          axis=1).astype(np.float32)
    FP8 = ml_dtypes.float8_e4m3fn
    b1rep = np.tile(b1, 4).astype(np.float32)
    b1hi = b1rep.astype(FP8)
    b1lo = (b1rep - b1hi.astype(np.float32)).astype(FP8)
    b8 = np.concatenate([np.asarray(b1hi), np.asarray(b1lo),
                         np.ones(256, np.float32).astype(FP8)]).reshape(1, -1)

    in_maps = []
    for k in range(N_CORES):
        lo, hi = int(bounds[k]), int(bounds[k + 1])
        cnt_rows = hi - lo
        xt = np.zeros((P, nrows), dtype=BF16)
        if cnt_rows:
            xt[:, :cnt_rows] = xbf[lo:hi].T
        blv = np.full(nrows, PAD_ID, dtype=np.float32)
        if cnt_rows:
            blv[:cnt_rows] = (batch[lo:hi] - k * SEGS).astype(np.float32)
        meta = np.ascontiguousarray(blv.reshape(nt, P).T).astype(BF16)
        crow = np.concatenate([
            counts_all[k * SEGS:(k + 1) * SEGS],
            b2, b3, b4, np.ones(P, np.float32),
        ]).reshape(1, -1).astype(np.float32)
        in_maps.append({
            "xt": xt, "meta": meta, "iex": iex_np,
            "w1": W1.astype(BF16),
            "b8": b8, "cmat": cmat, "crow": crow,
        })
    return nrows, in_maps


def kernel(x, batch, W1, b1, W2, b2, W3, b3, W4, b4):
    from concourse.bass_utils import run_bass_kernel_spmd

    nrows, in_maps = _prep_in_maps(x, batch, W1, b1, W2, b2, W3, b3, W4, b4)
    nc = _get_nc(nrows)
    res = run_bass_kernel_spmd(nc, in_maps, list(range(N_CORES)))
    out = np.concatenate([r["out"] for r in res.results], axis=0)
    return out.astype(np.float32)
